# revision 1
# baseline (speedup 1.0000x reference)
"""Trainium2 Bass kernel for nn_BCMEmulator (TCN emulator).

Model: 5-block dilated-causal-conv TCN (CH=64, K=3, dils 1,2,4,8,16) over
(B=128, T=1024) + pointwise heads (pet/pck softplus, aet sigmoid gate, cwd).

Strategy (pure data parallel, 8 cores x 16 sequences):
 - Each core processes 16 sequences as 8 "pairs". A pair packs 2 sequences
   into the 128 SBUF partitions: rows 0-63 = seq A channels, 64-127 = seq B.
 - Every conv tap is one matmul (K=128 contraction = 2x64 channels,
   block-diagonal weights, M=128 = 2x64 output channels, N=512 time cols).
   Causal dilation is a column offset into a left-zero-padded SBUF tensor.
 - float32r matmuls: 1 PE cycle/row for N>=256 (bf16 speed), ~1.3e-4 rel err.
 - ReLU on ScalarE (free per-partition bias), residual add fused on VectorE
   via scalar_tensor_tensor (f = max(psB,0) + f) when biases are zero.
 - softplus = ln(1+exp(.)), sigmoid(z) = exp(-ln(1+exp(-z))): only the
   natural_log_exp_and_others ACT table set is used (no table switches).
"""
import sys

sys.path.insert(0, "/opt/trn_rl_repo")

import numpy as np

import concourse.bacc as bacc
import concourse.bass as bass
import concourse.tile as tile
from concourse import mybir
from concourse.bass_utils import run_bass_kernel_spmd

B, T = 128, 1024
C_IN, EMB = 15, 8
CH = 64
DILS = [1, 2, 4, 8, 16]
CT = C_IN + EMB              # 23 input channels after fveg concat
NCORES = 8
BPC = B // NCORES            # 16 sequences per core
NPAIR = BPC // 2             # 8 pairs per core
P0 = 2 * DILS[-1]            # 32 left-pad columns (max lookback)
PADT = P0 + T
TT = 512                     # matmul free-dim tile (one PSUM bank of fp32)
NTT = T // TT

F32R = mybir.dt.float32r
F32 = mybir.dt.float32
AF = mybir.ActivationFunctionType
ALU = mybir.AluOpType

_PROGRAM_CACHE = {}


def _pin_act_table():
    """Force every ACT instruction onto natural_log_exp_and_others (which
    contains Relu+Exp+Ln): the greedy per-instruction set picker otherwise
    thrashes Relu/Exp->set0, Ln->set5, inserting ~33 table loads (~2.7us
    each, serializing ScalarE). Membership is edited, order preserved, so
    emitted act_func_set_ids still index act_info.json correctly."""
    import concourse.hw_specs as hw_specs
    if getattr(bacc.get_activation_tables, "_pinned", False):
        return
    orig = bacc.get_activation_tables
    mine = {AF.Relu, AF.Exp, AF.Ln}

    def patched(arch):
        tabs = orig(arch)
        return {
            name: (set(fns) if name == "natural_log_exp_and_others"
                   else set(fns) - mine)
            for name, fns in tabs.items()
        }

    patched._pinned = True
    bacc.get_activation_tables = patched
    hw_specs_patched = patched
    del hw_specs_patched


def build_program(zero_bb):
    _pin_act_table()
    """Build + compile the per-core Bass program.

    zero_bb: tuple of 4 bools — whether bb[i] is all-zero (enables the fused
    DVE relu+residual-add; otherwise an extra ACT relu-with-bias is emitted).
    """
    nc = bacc.Bacc("TRN2", target_bir_lowering=False, debug=False,
                   num_devices=NCORES)

    xin_d = nc.dram_tensor("xin", [NPAIR, 2 * CT, 1 + T], F32R, kind="ExternalInput")
    w0_d = nc.dram_tensor("w0", [4 * CT, 3, 128], F32R, kind="ExternalInput")
    wk_d = nc.dram_tensor("wk", [64, 27, 64], F32R, kind="ExternalInput")
    whp_d = nc.dram_tensor("whp", [128, 4], F32R, kind="ExternalInput")
    wha_d = nc.dram_tensor("wha", [128, 2], F32R, kind="ExternalInput")
    wh2_d = nc.dram_tensor("wh2", [4, 2], F32R, kind="ExternalInput")
    bias_d = nc.dram_tensor("bias", [128, 11], F32, kind="ExternalInput")
    bh_d = nc.dram_tensor("bh", [4, 2], F32, kind="ExternalInput")
    out_d = {
        nm: nc.dram_tensor(nm, [BPC, T], F32, kind="ExternalOutput")
        for nm in ("pet", "pck", "aet", "cwd")
    }

    with tile.TileContext(nc) as tc:
        with (
            tc.tile_pool(name="wpool", bufs=1) as wpool,
            tc.tile_pool(name="xpool", bufs=3) as xpool,
            tc.tile_pool(name="fpool", bufs=6) as fpool,
            tc.tile_pool(name="hpool", bufs=6) as hpool,
            tc.tile_pool(name="spool", bufs=2) as spool,
            tc.tile_pool(name="pspool", bufs=1, space=bass.MemorySpace.PSUM) as ps,
        ):
            w0_sb = wpool.tile([4 * CT, 3, 128], F32R)
            wk_sb = wpool.tile([128, 27, 128], F32R)
            whp_sb = wpool.tile([128, 4], F32R)
            wha_sb = wpool.tile([128, 2], F32R)
            wh2_sb = wpool.tile([4, 2], F32R)
            bias_sb = wpool.tile([128, 11], F32)
            bh_sb = wpool.tile([4, 2], F32)
            nc.gpsimd.dma_start(out=w0_sb, in_=w0_d[:])
            # wk ships as (64,27,64); the block-diagonal (128,27,128) lhsT is
            # assembled on-device: zero the tile once, then 2 DMAs fill the
            # diagonal quadrants (weights are static, so this runs once)
            nc.vector.memset(wk_sb.bitcast(F32), 0.0)

            # preload all pair inputs; the big wk DMA is issued after the
            # first pairs' xin so pair 0's conv0a isn't queued behind it
            xins = []
            for p in range(NPAIR):
                xin_sb = xpool.tile([4 * CT, T], F32R, tag=f"xin{p}",
                                    name=f"xin_sb{p}", bufs=1)
                # rows 0-45: xin; rows 46-91: xin right-shifted by one column
                # (the dram copy has one leading zero column, so the shifted
                # view is just the same dram region starting one col earlier)
                nc.sync.dma_start(out=xin_sb[0:2 * CT, :],
                                  in_=xin_d[p, :, 1:1 + T])
                eng = nc.gpsimd if p < 2 else nc.sync
                eng.dma_start(out=xin_sb[2 * CT:4 * CT, :],
                              in_=xin_d[p, :, 0:T])
                xins.append(xin_sb)
                if p == 0:
                    nc.sync.dma_start(out=bias_sb, in_=bias_d[:])
                if p == 1:
                    nc.sync.dma_start(out=bh_sb, in_=bh_d[:])
                    nc.sync.dma_start(out=wk_sb[0:64, :, 0:64], in_=wk_d[:])
                    nc.sync.dma_start(out=wk_sb[64:128, :, 64:128], in_=wk_d[:])
                    nc.sync.dma_start(out=whp_sb, in_=whp_d[:])
                    nc.sync.dma_start(out=wha_sb, in_=wha_d[:])
                    nc.sync.dma_start(out=wh2_sb, in_=wh2_d[:])

            def conv_taps(psum, lhsT_of_j, src, d, base_k):
                """3-tap dilated causal conv: psum += sum_j W_j @ src shifted
                right by s=(2-j)*d. Causal zero-padding falls out of PSUM
                has_written semantics: tap0 (start=True) clears the bank and
                writes only cols [s0:TT]; later taps overwrite the still-
                unwritten left edge and accumulate elsewhere."""
                for j in (2, 1, 0):
                    s = (2 - j) * d
                    for t in range(NTT):
                        lo = t * TT
                        out_lo = lo + (s if t == 0 else 0)
                        nc.tensor.matmul(
                            psum[:, out_lo:lo + TT],
                            lhsT_of_j(j),
                            src[:base_k, out_lo - s:lo + TT - s],
                            start=(j == 2),
                            stop=(j == 0),
                        )

            GRP = 2

            def conv_taps(psum, lhsT_of_j, src, d, base_k=128):
                """3-tap dilated causal conv: psum += sum_j W_j @ src shifted
                right by s=(2-j)*d. Causal zero-padding falls out of PSUM
                has_written semantics: the shift-0 tap goes first (start=True,
                full width, clears the bank); shifted taps then accumulate
                into fully-written regions, leaving the left edge untouched
                where their input would be out of range."""
                for j in (2, 1, 0):
                    s = (2 - j) * d
                    for t in range(NTT):
                        lo = t * TT
                        out_lo = lo + (s if t == 0 else 0)
                        nc.tensor.matmul(
                            psum[:, out_lo:lo + TT],
                            lhsT_of_j(j),
                            src[:base_k, out_lo - s:lo + TT - s],
                            start=(j == 2),
                            stop=(j == 0),
                        )

            # Two-level software pipeline, stage-interleaved emission:
            #  - pairs are processed in groups of GRP=2; within each stage the
            #    per-pair ops are emitted round-robin so every engine has
            #    same-stage work from both pairs queued;
            #  - the previous group's head stages are drained between the
            #    current group's block phases, so head transcendentals overlap
            #    the next group's conv matmuls and only the last group's heads
            #    form the kernel tail.
            # PSUM: tag p%4 (one (128,1024) = 2-bank slot per pair), so
            # consecutive groups use disjoint tag pairs {0,1}/{2,3}.
            st = {}

            def blk0_convA(grp):
                for p in grp:
                    psA = ps.tile([128, T], F32, tag=f"ps{p % 4}",
                                  name=f"psA0_{p}")
                    for t in range(NTT):
                        lo = t * TT
                        nc.tensor.matmul(psA[:, lo:lo + TT], w0_sb[0:92, 0, :],
                                         xins[p][0:92, lo:lo + TT],
                                         start=True, stop=False)
                    for t in range(NTT):
                        lo = t * TT
                        out_lo = lo + (2 if t == 0 else 0)
                        nc.tensor.matmul(psA[:, out_lo:lo + TT],
                                         w0_sb[0:46, 1, :],
                                         xins[p][0:46, out_lo - 2:lo + TT - 2],
                                         start=False, stop=True)
                    st[p] = {"ps": psA}
                for p in grp:
                    h1 = hpool.tile([128, T], F32R, tag="h1", name=f"h1b0_{p}")
                    nc.scalar.activation(out=h1, in_=st[p]["ps"], func=AF.Relu,
                                         bias=bias_sb[:, 0:1], scale=1.0)
                    st[p]["h1"] = h1
                    h1s = hpool.tile([128, T], F32R, tag="h1s", name=f"h1s_{p}")
                    nc.vector.tensor_scalar(out=h1s[:, 1:T],
                                            in0=st[p]["ps"][:, 0:T - 1],
                                            scalar1=bias_sb[:, 0:1],
                                            scalar2=0.0,
                                            op0=ALU.add, op1=ALU.max)
                    nc.vector.tensor_scalar_mul(out=h1s[:, 0:1],
                                                in0=bias_sb[:, 0:1],
                                                scalar1=0.0)
                    st[p]["h1s"] = h1s

            def blk0_convB(grp):
                for p in grp:
                    psB = ps.tile([128, T], F32, tag=f"ps{p % 4}",
                                  name=f"psB0_{p}")
                    h1, h1s = st[p]["h1"], st[p]["h1s"]
                    for t in range(NTT):
                        lo = t * TT
                        nc.tensor.matmul(psB[:, lo:lo + TT], wk_sb[:, 2, :],
                                         h1[:, lo:lo + TT],
                                         start=True, stop=False)
                    for t in range(NTT):
                        lo = t * TT
                        nc.tensor.matmul(psB[:, lo:lo + TT], wk_sb[:, 1, :],
                                         h1s[:, lo:lo + TT],
                                         start=False, stop=False)
                    for t in range(NTT):
                        lo = t * TT
                        out_lo = lo + (2 if t == 0 else 0)
                        nc.tensor.matmul(psB[:, out_lo:lo + TT], wk_sb[:, 0, :],
                                         h1[:, out_lo - 2:lo + TT - 2],
                                         start=False, stop=True)
                    st[p]["ps"] = psB
                for p in grp:
                    h2 = hpool.tile([128, T], F32, tag="h2", name=f"h2_{p}",
                                    bufs=5)
                    nc.vector.tensor_scalar(out=h2, in0=st[p]["ps"],
                                            scalar1=bias_sb[:, 1:2],
                                            scalar2=0.0,
                                            op0=ALU.add, op1=ALU.max)
                    st[p]["h2"] = h2

            def blk0_resid(grp):
                for p in grp:
                    psR = ps.tile([128, T], F32, tag=f"ps{p % 4}",
                                  name=f"psR_{p}")
                    for t in range(NTT):
                        nc.tensor.matmul(
                            psR[:, t * TT:(t + 1) * TT], w0_sb[0:46, 2, :],
                            xins[p][0:46, t * TT:(t + 1) * TT],
                            start=True, stop=True)
                    st[p]["ps"] = psR
                for p in grp:
                    f = fpool.tile([128, T], F32R, tag="f", name=f"f_{p}")
                    nc.vector.scalar_tensor_tensor(
                        out=f, in0=st[p]["ps"], scalar=bias_sb[:, 2:3],
                        in1=st[p]["h2"], op0=ALU.add, op1=ALU.add)
                    st[p]["f"] = f

            def blk(grp, i, d):
                for p in grp:
                    psA = ps.tile([128, T], F32, tag=f"ps{p % 4}",
                                  name=f"psA{i + 1}_{p}")
                    conv_taps(psA, lambda j: wk_sb[:, 3 + 6 * i + j, :],
                              st[p]["f"], d)
                    st[p]["ps"] = psA
                for p in grp:
                    h1 = hpool.tile([128, T], F32R, tag="h1",
                                    name=f"h1_{i + 1}_{p}")
                    nc.scalar.activation(out=h1, in_=st[p]["ps"], func=AF.Relu,
                                         bias=bias_sb[:, 3 + i:4 + i],
                                         scale=1.0)
                    st[p]["h1"] = h1
                for p in grp:
                    psB = ps.tile([128, T], F32, tag=f"ps{p % 4}",
                                  name=f"psB{i + 1}_{p}")
                    conv_taps(psB, lambda j: wk_sb[:, 6 + 6 * i + j, :],
                              st[p]["h1"], d)
                    st[p]["ps"] = psB
                for p in grp:
                    if zero_bb[i]:
                        nc.vector.scalar_tensor_tensor(
                            out=st[p]["f"], in0=st[p]["ps"], scalar=0.0,
                            in1=st[p]["f"].bitcast(F32),
                            op0=ALU.max, op1=ALU.add)
                    else:
                        h2 = hpool.tile([128, T], F32, tag="h2",
                                        name=f"h2_{i + 1}_{p}", bufs=5)
                        nc.scalar.activation(out=h2, in_=st[p]["ps"],
                                             func=AF.Relu,
                                             bias=bias_sb[:, 7 + i:8 + i],
                                             scale=1.0)
                        nc.vector.tensor_tensor(
                            out=st[p]["f"], in0=st[p]["f"].bitcast(F32),
                            in1=h2, op=ALU.add)

            def head_stages(grp):
                def s_mmH():
                    for p in grp:
                        psH = ps.tile([4, T], F32, tag=f"ps{p % 4}",
                                      name=f"psH_{p}")
                        for t in range(NTT):
                            sl = slice(t * TT, (t + 1) * TT)
                            nc.tensor.matmul(psH[:, sl], whp_sb,
                                             st[p]["f"][:, sl],
                                             start=True, stop=True)
                        st[p]["ps"] = psH

                def s_spe():
                    for p in grp:
                        spe = spool.tile([4, T], F32, tag="spe",
                                         name=f"spe_{p}")
                        nc.scalar.activation(out=spe, in_=st[p]["ps"],
                                             func=AF.Exp,
                                             bias=bh_sb[:, 0:1], scale=1.0)
                        st[p]["spe"] = spe

                def s_sp():
                    for p in grp:
                        sp = spool.tile([4, T], F32R, tag="sp",
                                        name=f"sp_{p}", bufs=5)
                        nc.scalar.activation(out=sp, in_=st[p]["spe"],
                                             func=AF.Ln, bias=1.0, scale=1.0)
                        st[p]["sp"] = sp

                def s_mmH2():
                    for p in grp:
                        psH2 = ps.tile([2, T], F32, tag=f"ps{p % 4}",
                                       name=f"psH2_{p}")
                        for t in range(NTT):
                            sl = slice(t * TT, (t + 1) * TT)
                            nc.tensor.matmul(psH2[:, sl], wha_sb,
                                             st[p]["f"][:, sl],
                                             start=True, stop=False)
                        for t in range(NTT):
                            sl = slice(t * TT, (t + 1) * TT)
                            nc.tensor.matmul(psH2[:, sl], wh2_sb,
                                             st[p]["sp"][:, sl],
                                             start=False, stop=True)
                        st[p]["ps"] = psH2

                def s_ge():
                    for p in grp:
                        ge = spool.tile([2, T], F32, tag="ge", name=f"ge_{p}")
                        nc.scalar.activation(out=ge, in_=st[p]["ps"],
                                             func=AF.Exp,
                                             bias=bh_sb[0:2, 1:2], scale=-1.0)
                        st[p]["ge"] = ge

                def s_gl():
                    for p in grp:
                        gl = spool.tile([2, T], F32, tag="gl", name=f"gl_{p}")
                        nc.scalar.activation(out=gl, in_=st[p]["ge"],
                                             func=AF.Ln, bias=1.0, scale=1.0)
                        st[p]["gl"] = gl

                def s_gg():
                    for p in grp:
                        gg = spool.tile([2, T], F32, tag="gg", name=f"gg_{p}")
                        nc.scalar.activation(out=gg, in_=st[p]["gl"],
                                             func=AF.Exp, scale=-1.0)
                        st[p]["gg"] = gg

                def s_petdma():
                    for p in grp:
                        nc.sync.dma_start(
                            out=out_d["pet"][2 * p:2 * p + 2, :],
                            in_=st[p]["sp"].bitcast(F32)[0:2, :])
                        nc.sync.dma_start(
                            out=out_d["pck"][2 * p:2 * p + 2, :],
                            in_=st[p]["sp"].bitcast(F32)[2:4, :])

                def s_aet():
                    for p in grp:
                        aet = spool.tile([2, T], F32, tag="aet",
                                         name=f"aet_{p}")
                        nc.vector.tensor_tensor(
                            out=aet, in0=st[p]["gg"],
                            in1=st[p]["sp"].bitcast(F32)[0:2, :], op=ALU.mult)
                        st[p]["aet"] = aet

                def s_cwd():
                    for p in grp:
                        cwd = spool.tile([2, T], F32, tag="cwd",
                                         name=f"cwd_{p}")
                        nc.vector.tensor_tensor(
                            out=cwd, in0=st[p]["sp"].bitcast(F32)[0:2, :],
                            in1=st[p]["aet"], op=ALU.subtract)
                        st[p]["cwd"] = cwd

                def s_aetdma():
                    for p in grp:
                        nc.sync.dma_start(
                            out=out_d["aet"][2 * p:2 * p + 2, :],
                            in_=st[p]["aet"])

                def s_cwddma():
                    for p in grp:
                        nc.sync.dma_start(
                            out=out_d["cwd"][2 * p:2 * p + 2, :],
                            in_=st[p]["cwd"])

                return [s_mmH, s_spe, s_sp, s_petdma, s_mmH2, s_ge, s_gl,
                        s_gg, s_aet, s_aetdma, s_cwd, s_cwddma]

            pending = []

            def drain(n):
                for _ in range(n):
                    if pending:
                        pending.pop(0)()

            for g0 in range(0, NPAIR, GRP):
                grp = list(range(g0, min(g0 + GRP, NPAIR)))
                phases = [lambda: blk0_convA(grp), lambda: blk0_convB(grp),
                          lambda: blk0_resid(grp)]
                for i, d in enumerate(DILS[1:]):
                    phases.append(lambda i=i, d=d: blk(grp, i, d))
                for ph in phases:
                    ph()
                    drain(2)
                drain(len(pending))
                pending = head_stages(grp)
            for s in pending:
                s()

    nc.compile()
    return nc


def get_program(zero_bb):
    key = tuple(zero_bb)
    if key not in _PROGRAM_CACHE:
        _PROGRAM_CACHE[key] = build_program(key)
    return _PROGRAM_CACHE[key]


def prep_inputs(inputs):
    """Host-side packing: returns (zero_bb, shared weight map, per-core xin)."""
    g = {k: np.asarray(v) for k, v in inputs.items()}
    x = g["x"].astype(np.float32, copy=False)
    ids = g["fveg_ids"].astype(np.int64)
    emb = g["fveg_emb"].astype(np.float32, copy=False)

    fv = emb[ids]                                     # (B, EMB)
    xin = np.concatenate(
        [x, np.broadcast_to(fv[:, :, None], (B, EMB, T))], axis=1)  # (B,23,T)
    xin_pad = np.zeros((B, CT, 1 + T), np.float32)
    xin_pad[:, :, 1:] = xin
    xin_cores = np.ascontiguousarray(
        xin_pad.reshape(NCORES, NPAIR, 2 * CT, 1 + T))

    w0 = np.zeros((4 * CT, 3, 128), np.float32)
    w0a, w0r = g["w0a"].astype(np.float32), g["w0r"].astype(np.float32)
    for s in range(2):                  # seq-in-pair
        r0, c0 = s * CT, s * 64
        w0[r0:r0 + CT, 0, c0:c0 + 64] = w0a[:, :, 2].T          # tap2, x
        w0[46 + r0:46 + r0 + CT, 0, c0:c0 + 64] = w0a[:, :, 1].T  # tap1, xsh
        w0[r0:r0 + CT, 1, c0:c0 + 64] = w0a[:, :, 0].T          # tap0
        w0[r0:r0 + CT, 2, c0:c0 + 64] = w0r[:, :, 0].T          # 1x1 resid
    

    wk = np.zeros((27, 64, 64), np.float32)
    for j in range(3):
        wk[j] = g["w0b"].astype(np.float32)[:, :, j].T
    wa, wb = g["wa"].astype(np.float32), g["wb"].astype(np.float32)
    for i in range(4):
        for j in range(3):
            wk[3 + 6 * i + j] = wa[i, :, :, j].T
            wk[6 + 6 * i + j] = wb[i, :, :, j].T
    wk = np.ascontiguousarray(wk.transpose(1, 0, 2))  # (64, 27, 64)

    pet_w = g["pet_w"].astype(np.float32)[0, :, 0]    # (64,)
    pck_w = g["pck_w"].astype(np.float32)[0, :, 0]
    aet_w = g["aet_w"].astype(np.float32)[0, :, 0]    # (66,)
    whp = np.zeros((128, 4), np.float32)
    whp[0:64, 0] = pet_w
    whp[64:128, 1] = pet_w
    whp[0:64, 2] = pck_w
    whp[64:128, 3] = pck_w
    wha = np.zeros((128, 2), np.float32)
    wha[0:64, 0] = aet_w[0:64]
    wha[64:128, 1] = aet_w[0:64]
    wpet, wpck = aet_w[64], aet_w[65]
    wh2 = np.array([[wpet, 0], [0, wpet], [wpck, 0], [0, wpck]], np.float32)

    bcols = [g["b0a"], g["b0b"], g["b0r"]] + [g["ba"][i] for i in range(4)] \
        + [g["bb"][i] for i in range(4)]
    bias = np.stack([np.tile(c.astype(np.float32), 2) for c in bcols], axis=1)

    pet_b = float(g["pet_b"][0])
    pck_b = float(g["pck_b"][0])
    aet_b = float(g["aet_b"][0])
    bh = np.array([[pet_b, -aet_b], [pet_b, -aet_b],
                   [pck_b, 0.0], [pck_b, 0.0]], np.float32)

    zero_bb = tuple(bool(np.all(g["bb"][i] == 0)) for i in range(4))
    shared = {"w0": w0, "wk": wk, "whp": whp, "wha": wha, "wh2": wh2,
              "bias": bias, "bh": bh}
    return zero_bb, shared, xin_cores


def run(inputs, trace=False, trace_kwargs=None):
    zero_bb, shared, xin_cores = prep_inputs(inputs)
    nc = get_program(zero_bb)
    in_maps = [
        {"xin": np.ascontiguousarray(xin_cores[c]), **shared}
        for c in range(NCORES)
    ]
    res = run_bass_kernel_spmd(nc, in_maps, core_ids=list(range(NCORES)),
                               trace=trace, **(trace_kwargs or {}))
    outs = []
    for nm in ("pet", "pck", "aet", "cwd"):
        full = np.concatenate([res.results[c][nm] for c in range(NCORES)], 0)
        outs.append(full.reshape(B, 1, T).astype(np.float32))
    return tuple(outs), res


def kernel(**inputs):
    outs, _ = run(inputs)
    return outs


def build_calib():
    """Same I/O signature as the real program, minimal compute — used by the
    bench to measure the axon relay's per-exec input-staging overhead."""
    _pin_act_table()
    nc = bacc.Bacc("TRN2", target_bir_lowering=False, debug=False,
                   num_devices=NCORES)
    xin_d = nc.dram_tensor("xin", [NPAIR, 2 * CT, 1 + T], F32R,
                           kind="ExternalInput")
    w0_d = nc.dram_tensor("w0", [4 * CT, 3, 128], F32R, kind="ExternalInput")
    wk_d = nc.dram_tensor("wk", [64, 27, 64], F32R, kind="ExternalInput")
    whp_d = nc.dram_tensor("whp", [128, 4], F32R, kind="ExternalInput")
    wha_d = nc.dram_tensor("wha", [128, 2], F32R, kind="ExternalInput")
    wh2_d = nc.dram_tensor("wh2", [4, 2], F32R, kind="ExternalInput")
    bias_d = nc.dram_tensor("bias", [128, 11], F32, kind="ExternalInput")
    bh_d = nc.dram_tensor("bh", [4, 2], F32, kind="ExternalInput")
    out_d = {
        nm: nc.dram_tensor(nm, [BPC, T], F32, kind="ExternalOutput")
        for nm in ("pet", "pck", "aet", "cwd")
    }
    with tile.TileContext(nc) as tc:
        with tc.tile_pool(name="sb", bufs=2) as sb:
            t = sb.tile([BPC, T], F32)
            nc.sync.dma_start(out=t, in_=xin_d.bitcast(F32)[0, 0:BPC, 1:1 + T])
            for nm in ("pet", "pck", "aet", "cwd"):
                nc.sync.dma_start(out=out_d[nm][:], in_=t)
    nc.compile()
    return nc



# revision 2
# speedup vs baseline: 1.2614x; 1.2614x over previous
"""Trainium2 Bass kernel for nn_BCMEmulator (TCN emulator) — fp8 DoubleRow rev.

Model: 5-block dilated-causal-conv TCN (CH=64, K=3, dils 1,2,4,8,16) over
(B=128, T=1024) + pointwise heads (pet/pck softplus, aet sigmoid gate, cwd).

Strategy (pure data parallel, 8 cores x 16 sequences; 2 seqs packed per
128 SBUF partitions = "pair", block-diagonal weights):
 - conv0a + 1x1 residual stay f32r (accuracy-critical first layer).
 - conv0b and all 4 residual blocks run in fp8-e4m3 with DoubleRow matmuls:
   one PE instruction contracts 2 k-tiles (2 conv taps) — taps are shifted
   views of a single zero-left-margin SBUF tile, expressed as a 3-dim access
   pattern [K, 2, N] whose middle dim strides by the dilation.
 - f (residual trunk) is kept as an exact f32r master (DVE relu+add update
   from PSUM); fp8 shadows for the next block's conv are cast on the
   otherwise-idle GPSIMD engine. Numerically validated: worst rel err
   ~1.1e-2 vs the 2e-2 gate (fp8 in block0's conv0b + blocks 1-4 only).
 - Heads are batched across all 8 pairs: per-pair 1x1 matmuls land in a
   shared (128,1024) PSUM tile (pet/pck logit rows 4p..4p+3, gate rows
   64+4p..); softplus/sigmoid chains then run as (32,1024) ACT passes
   (exp/ln only — single activation table) and (32,1024) DVE ops in bf16.
   Outputs ship as bf16 and are upcast on the host.
"""
import sys

sys.path.insert(0, "/opt/trn_rl_repo")

import numpy as np
import ml_dtypes

import concourse.bacc as bacc
import concourse.bass as bass
import concourse.tile as tile
from concourse import mybir
from concourse.bass_utils import run_bass_kernel_spmd

B, T = 128, 1024
C_IN, EMB = 15, 8
CH = 64
DILS = [1, 2, 4, 8, 16]
CT = C_IN + EMB              # 23 input channels after fveg concat
NCORES = 8
BPC = B // NCORES            # 16 sequences per core
NPAIR = BPC // 2             # 8 pairs per core
TT = 512                     # matmul free-dim tile (one PSUM bank of fp32)
NTT = T // TT
MARG = 32                    # left zero margin for block conv tiles (2*16)
M0 = 2                       # left zero margin for blk0 tiles

F32R = mybir.dt.float32r
F32 = mybir.dt.float32
FP8 = mybir.dt.float8e4
BF16 = mybir.dt.bfloat16
E4 = ml_dtypes.float8_e4m3
BF = ml_dtypes.bfloat16
AF = mybir.ActivationFunctionType
ALU = mybir.AluOpType
DRM = mybir.MatmulPerfMode.DoubleRow

NU = 18  # fp8 DR lhsT units: conv0b 2 + 4 blocks * (convA 2 + convB 2)

_PROGRAM_CACHE = {}


def _pin_act_table():
    """Force every ACT instruction onto natural_log_exp_and_others (contains
    Relu+Exp+Ln+Copy): avoids per-instruction table thrash (~2.7us a load)."""
    if getattr(bacc.get_activation_tables, "_pinned", False):
        return
    orig = bacc.get_activation_tables
    mine = {AF.Relu, AF.Exp, AF.Ln, AF.Copy}

    def patched(arch):
        tabs = orig(arch)
        return {
            name: (set(fns) if name == "natural_log_exp_and_others"
                   else set(fns) - mine)
            for name, fns in tabs.items()
        }

    patched._pinned = True
    bacc.get_activation_tables = patched


def dr_rhs(t_ap, base_col, d, n, k=128):
    """DoubleRow rhs AP [k, 2, n] on a 2-D tile: slot0 = cols shifted left by
    d (tap with shift d), slot1 = unshifted (tap with shift 0)."""
    pitch = t_ap.ap[0][0]
    return bass.AP(t_ap.tensor, t_ap.offset + base_col - d,
                   [[pitch, k], [d, 2], [1, n]])


def build_program(flags, reps=1):
    """flags: (zb0a, zb0b0r, zba, zbb, zheads) all-zero-bias fast paths."""
    _pin_act_table()
    zb0a, zb0b0r, zba, zbb, zheads = flags
    nc = bacc.Bacc("TRN2", target_bir_lowering=False, debug=False,
                   num_devices=NCORES)

    xin_d = nc.dram_tensor("xin", [NPAIR, 2 * CT, M0 + T], F32R,
                           kind="ExternalInput")
    w0_d = nc.dram_tensor("w0", [4 * CT, 3, 128], F32R, kind="ExternalInput")
    wk_d = nc.dram_tensor("wk", [128, 2 * NU, 128], FP8, kind="ExternalInput")
    wh_d = nc.dram_tensor("wh", [128, NPAIR, 96], F32R,
                          kind="ExternalInput")
    wsp_d = nc.dram_tensor("wsp", [32, 32], BF16, kind="ExternalInput")
    bias_d = nc.dram_tensor("bias", [128, 11], F32, kind="ExternalInput")
    bh_d = nc.dram_tensor("bh", [32, 2], F32, kind="ExternalInput")
    out_d = {
        nm: nc.dram_tensor(nm, [BPC, T], BF16, kind="ExternalOutput")
        for nm in ("pet", "pck", "aet", "cwd")
    }

    with tile.TileContext(nc) as tc:
        with (
            tc.tile_pool(name="wpool", bufs=1) as wpool,
            tc.tile_pool(name="xpool", bufs=1) as xpool,
            tc.tile_pool(name="hpool", bufs=1) as hpool,
            tc.tile_pool(name="fpool", bufs=1) as fpool,
            tc.tile_pool(name="spool", bufs=1) as spool,
            tc.tile_pool(name="pspool", bufs=1, space=bass.MemorySpace.PSUM) as ps,
        ):
            w0_sb = wpool.tile([4 * CT, 3, 128], F32R)
            wk_sb = wpool.tile([128, 2 * NU, 128], FP8)
            wh_sb = wpool.tile([128, NPAIR, 96], F32R)
            wsp_sb = wpool.tile([32, 32], BF16)
            bias_sb = wpool.tile([128, 11], F32)
            bh_sb = wpool.tile([32, 2], F32)
            nc.sync.dma_start(out=w0_sb, in_=w0_d[:])
            nc.gpsimd.dma_start(out=wk_sb, in_=wk_d[:])
            nc.sync.dma_start(out=wh_sb, in_=wh_d[:])
            nc.sync.dma_start(out=wsp_sb, in_=wsp_d[:])
            nc.sync.dma_start(out=bias_sb, in_=bias_d[:])
            nc.sync.dma_start(out=bh_sb, in_=bh_d[:])

            # persistent data tiles
            xins = []
            for p in range(NPAIR):
                xt = xpool.tile([2 * 2 * CT, M0 + T], F32R, tag=f"xin{p}", name=f"xin_sb{p}")
                eng = nc.sync if p % 2 == 0 else nc.gpsimd
                eng.dma_start(out=xt[0:2 * CT, :], in_=xin_d[p, :, :])
                # col 0 of the shifted rows is never read (all views start
                # at col >= M0-2 on rows 0:46 or >= M0 on the packed rows)
                eng.dma_start(out=xt[2 * CT:4 * CT, 1:M0 + T],
                              in_=xin_d[p, :, 0:M0 + T - 1])
                xins.append(xt)
            h0t = []
            for k in range(2):
                t = hpool.tile([128, M0 + T], FP8, tag=f"h0_{k}", name=f"h0t{k}")
                nc.vector.memset(t[:, 0:M0], 0.0)
                h0t.append(t)
            h1t = []
            for k in range(4):
                t = hpool.tile([128, MARG + T], FP8, tag=f"h1_{k}", name=f"h1t{k}")
                nc.vector.memset(t[:, 0:MARG], 0.0)
                h1t.append(t)
            sht = []
            for k in range(4):
                t = hpool.tile([128, MARG + T], FP8, tag=f"sh_{k}", name=f"sht{k}")
                nc.vector.memset(t[:, 0:MARG], 0.0)
                sht.append(t)
            fts = [fpool.tile([128, T], F32R, tag=f"f{p}", name=f"fm{p}")
                   for p in range(NPAIR)]

            psc = [ps.tile([128, T], F32, tag=f"pc{i}", name=f"pc{i}")
                   for i in range(3)]
            psH = ps.tile([128, T], F32, tag="ph", name="psHEAD")

            st = {}

            def blk0_convA(grp):
                for p in grp:
                    psA = psc[p % 2]
                    for t in range(NTT):
                        lo = t * TT
                        nc.tensor.matmul(psA[:, lo:lo + TT], w0_sb[:, 0, :],
                                         xins[p][0:4 * CT, M0 + lo:M0 + lo + TT],
                                         start=True, stop=False)
                        nc.tensor.matmul(psA[:, lo:lo + TT],
                                         w0_sb[0:2 * CT, 1, :],
                                         xins[p][0:2 * CT, lo:lo + TT],
                                         start=False, stop=True)
                    st[p] = {"ps": psA}

            def blk0_relu(grp):
                for p in grp:
                    h0 = h0t[p % 2]
                    if zb0a:
                        nc.scalar.activation(out=h0[:, M0:M0 + T],
                                             in_=st[p]["ps"], func=AF.Relu,
                                             bias=0.0, scale=1.0)
                    else:
                        nc.scalar.activation(out=h0[:, M0:M0 + T],
                                             in_=st[p]["ps"], func=AF.Relu,
                                             bias=bias_sb[:, 0:1], scale=1.0)
                    st[p]["h0"] = h0

            def blk0_convB(grp):
                for p in grp:
                    h0 = st[p]["h0"][:, :]
                    psB = psc[p % 2]
                    for t in range(NTT):
                        lo = t * TT
                        nc.tensor.matmul(psB[:, lo:lo + TT], wk_sb[:, 0:2, :],
                                         dr_rhs(h0, M0 + lo, 1, TT),
                                         start=True, stop=False,
                                         perf_mode=DRM)
                        nc.tensor.matmul(psB[:, lo:lo + TT], wk_sb[:, 2:4, :],
                                         dr_rhs(h0, M0 + lo, 2, TT),
                                         start=False, stop=True,
                                         perf_mode=DRM)
                    st[p]["ps"] = psB

            def blk0_resid(grp):
                for p in grp:
                    psR = psc[2]
                    for t in range(NTT):
                        lo = t * TT
                        nc.tensor.matmul(psR[:, lo:lo + TT],
                                         w0_sb[0:2 * CT, 2, :],
                                         xins[p][0:2 * CT, M0 + lo:M0 + lo + TT],
                                         start=True, stop=True)
                    st[p]["psR"] = psR

            def blk0_f0(grp):
                # walrus allows only one PSUM operand per DVE instruction:
                # materialize relu(psB)+b0b on ACT first, then add psR on DVE
                for p in grp:
                    f = fts[p]
                    h2 = spool.tile([128, T], F32R, tag=f"h2_{p % 2}", bufs=1,
                                    name=f"h2tmp{p % 2}")
                    if zb0b0r:
                        nc.scalar.activation(out=h2, in_=st[p]["ps"],
                                             func=AF.Relu, bias=0.0, scale=1.0)
                        nc.vector.scalar_tensor_tensor(
                            out=f, in0=st[p]["psR"], scalar=0.0,
                            in1=h2, op0=ALU.add, op1=ALU.add)
                    else:
                        nc.scalar.activation(out=h2, in_=st[p]["ps"],
                                             func=AF.Relu,
                                             bias=bias_sb[:, 1:2], scale=1.0)
                        nc.vector.scalar_tensor_tensor(
                            out=f, in0=st[p]["psR"], scalar=bias_sb[:, 2:3],
                            in1=h2, op0=ALU.add, op1=ALU.add)
                    st[p]["f"] = f

            def cast(grp, k):
                for p in grp:
                    s = sht[2 * (p % 2) + k % 2]
                    nc.gpsimd.tensor_copy(out=s[:, MARG:MARG + T],
                                          in_=st[p]["f"])
                    st[p]["sh"] = s

            def blk_convA(grp, i, d):
                for p in grp:
                    sh = st[p]["sh"][:, :]
                    psA = psc[p % 2]
                    u = 2 * (2 + 4 * i)
                    for t in range(NTT):
                        lo = t * TT
                        nc.tensor.matmul(psA[:, lo:lo + TT],
                                         wk_sb[:, u:u + 2, :],
                                         dr_rhs(sh, MARG + lo, d, TT),
                                         start=True, stop=False,
                                         perf_mode=DRM)
                        nc.tensor.matmul(psA[:, lo:lo + TT],
                                         wk_sb[:, u + 2:u + 4, :],
                                         dr_rhs(sh, MARG + lo, 2 * d, TT),
                                         start=False, stop=True,
                                         perf_mode=DRM)
                    st[p]["ps"] = psA

            def blk_relu(grp, i):
                for p in grp:
                    h1 = h1t[2 * (p % 2) + i % 2]
                    if zba:
                        nc.scalar.activation(out=h1[:, MARG:MARG + T],
                                             in_=st[p]["ps"], func=AF.Relu,
                                             bias=0.0, scale=1.0)
                    else:
                        nc.scalar.activation(out=h1[:, MARG:MARG + T],
                                             in_=st[p]["ps"], func=AF.Relu,
                                             bias=bias_sb[:, 3 + i:4 + i],
                                             scale=1.0)
                    st[p]["h1"] = h1

            def blk_convB(grp, i, d):
                for p in grp:
                    h1 = st[p]["h1"][:, :]
                    psB = psc[p % 2]
                    u = 2 * (2 + 4 * i + 2)
                    for t in range(NTT):
                        lo = t * TT
                        nc.tensor.matmul(psB[:, lo:lo + TT],
                                         wk_sb[:, u:u + 2, :],
                                         dr_rhs(h1, MARG + lo, d, TT),
                                         start=True, stop=False,
                                         perf_mode=DRM)
                        nc.tensor.matmul(psB[:, lo:lo + TT],
                                         wk_sb[:, u + 2:u + 4, :],
                                         dr_rhs(h1, MARG + lo, 2 * d, TT),
                                         start=False, stop=True,
                                         perf_mode=DRM)
                    st[p]["ps"] = psB

            def blk_update(grp, i):
                for p in grp:
                    f = st[p]["f"]
                    if zbb:
                        nc.vector.scalar_tensor_tensor(
                            out=f, in0=st[p]["ps"], scalar=0.0,
                            in1=f.bitcast(F32), op0=ALU.max, op1=ALU.add)
                    else:
                        h2 = spool.tile([128, T], F32, tag="h2tmp", bufs=2, name="h2tmp")
                        nc.scalar.activation(out=h2, in_=st[p]["ps"],
                                             func=AF.Relu,
                                             bias=bias_sb[:, 7 + i:8 + i],
                                             scale=1.0)
                        nc.vector.tensor_tensor(out=f, in0=f.bitcast(F32),
                                                in1=h2, op=ALU.add)

            def head_mm(grp):
                # one accumulating matmul per pair per col-bank writes pet/pck
                # logit rows 0:32 AND gate logit rows 64:96 (lhsT zero rows
                # 32:64 between) — a single pending PSUM group per bank that
                # the tail's wsp matmul closes. Pair p's lhsT slice is zero
                # except out cols 4p..4p+3 (and 64+4p..).
                for p in grp:
                    f = st[p]["f"]
                    for t in range(NTT):
                        lo = t * TT
                        nc.tensor.matmul(psH[0:96, lo:lo + TT],
                                         wh_sb[:, p, :], f[:, lo:lo + TT],
                                         start=(p == 0),
                                         stop=(p == NPAIR - 1))

            for rep in range(reps):
                for g0 in range(0, NPAIR, 2):
                    grp = [g0, g0 + 1]
                    blk0_convA(grp)
                    blk0_relu(grp)
                    blk0_resid([grp[0]])
                    blk0_convB(grp)
                    blk0_f0([grp[0]])
                    blk0_resid([grp[1]])
                    blk0_f0([grp[1]])
                    cast(grp, 0)
                    for i, d in enumerate(DILS[1:]):
                        blk_convA(grp, i, d)
                        blk_relu(grp, i)
                        blk_convB(grp, i, d)
                        blk_update(grp, i)
                        if i < 3:
                            cast(grp, i + 1)
                    head_mm(grp)

                # ---- batched tail ----
                spe = spool.tile([32, T], BF16, tag="spe")
                sp = spool.tile([32, T], BF16, tag="sp")
                ge = spool.tile([32, T], F32, tag="ge")
                gl = spool.tile([32, T], F32, tag="gl")
                gg = spool.tile([32, T], F32, tag="gg")
                aet = spool.tile([32, T], BF16, tag="aet")
                cwd = spool.tile([32, T], BF16, tag="cwd")
                e1 = spool.tile([32, T], F32, tag="e1", name="e1t")
                for t in range(NTT):
                    sl = slice(t * TT, (t + 1) * TT)
                    if zheads:
                        nc.scalar.activation(out=spe[:, sl], in_=psH[0:32, sl],
                                             func=AF.Exp, bias=0.0, scale=1.0)
                    else:
                        nc.scalar.activation(out=spe[:, sl], in_=psH[0:32, sl],
                                             func=AF.Exp, bias=bh_sb[:, 0:1],
                                             scale=1.0)
                    nc.scalar.activation(out=sp[:, sl], in_=spe[:, sl],
                                         func=AF.Ln, bias=1.0, scale=1.0)
                    # gate logit is split: z1 = wha part (psH rows 64:96),
                    # z2 = rank-4 softplus part (free conv slot); only one
                    # PSUM operand is allowed per DVE op, so combine
                    # multiplicatively: exp(-(z1+z2)) = exp(-z1)*exp(-z2)
                    nc.tensor.matmul(psc[0][0:32, sl], wsp_sb, sp[0:32, sl],
                                     start=True, stop=True)
                    if zheads:
                        nc.scalar.activation(out=e1[:, sl],
                                             in_=psH[64:96, sl],
                                             func=AF.Exp, bias=0.0, scale=-1.0)
                    else:
                        nc.scalar.activation(out=e1[:, sl],
                                             in_=psH[64:96, sl],
                                             func=AF.Exp, bias=bh_sb[:, 1:2],
                                             scale=-1.0)
                    nc.scalar.activation(out=ge[:, sl], in_=psc[0][0:32, sl],
                                         func=AF.Exp, bias=0.0, scale=-1.0)
                    nc.vector.tensor_tensor(out=ge[:, sl], in0=ge[:, sl],
                                            in1=e1[:, sl], op=ALU.mult)
                    nc.scalar.activation(out=gl[:, sl], in_=ge[:, sl],
                                         func=AF.Ln, bias=1.0, scale=1.0)
                    nc.scalar.activation(out=gg[:, sl], in_=gl[:, sl],
                                         func=AF.Exp, scale=-1.0)
                    nc.vector.tensor_tensor(out=aet[:, sl], in0=gg[:, sl],
                                            in1=sp[:, sl], op=ALU.mult)
                    nc.vector.tensor_tensor(out=cwd[:, sl], in0=sp[:, sl],
                                            in1=aet[:, sl], op=ALU.subtract)
                for p in range(NPAIR):
                    nc.sync.dma_start(out=out_d["pet"][2 * p:2 * p + 2, :],
                                      in_=sp[4 * p:4 * p + 2, :])
                    nc.sync.dma_start(out=out_d["pck"][2 * p:2 * p + 2, :],
                                      in_=sp[4 * p + 2:4 * p + 4, :])
                    nc.sync.dma_start(out=out_d["aet"][2 * p:2 * p + 2, :],
                                      in_=aet[4 * p:4 * p + 2, :])
                    nc.sync.dma_start(out=out_d["cwd"][2 * p:2 * p + 2, :],
                                      in_=cwd[4 * p:4 * p + 2, :])

    nc.compile()
    return nc


def get_program(flags, reps=1):
    key = (tuple(flags), reps)
    if key not in _PROGRAM_CACHE:
        _PROGRAM_CACHE[key] = build_program(tuple(flags), reps)
    return _PROGRAM_CACHE[key]


def prep_inputs(inputs):
    """Host-side packing: returns (flags, shared weight map, per-core xin)."""
    g = {k: np.asarray(v) for k, v in inputs.items()}
    x = g["x"].astype(np.float32, copy=False)
    ids = g["fveg_ids"].astype(np.int64)
    emb = g["fveg_emb"].astype(np.float32, copy=False)

    fv = emb[ids]                                     # (B, EMB)
    xin = np.concatenate(
        [x, np.broadcast_to(fv[:, :, None], (B, EMB, T))], axis=1)  # (B,23,T)
    xin_pad = np.zeros((B, CT, M0 + T), np.float32)
    xin_pad[:, :, M0:] = xin
    xin_cores = np.ascontiguousarray(
        xin_pad.reshape(NCORES, NPAIR, 2 * CT, M0 + T))

    # f32r conv0a + resid weights: slot0 = taps(2,1) packed on 92 rows,
    # slot1 = tap0 (46 rows), slot2 = resid 1x1 (46 rows)
    w0 = np.zeros((4 * CT, 3, 128), np.float32)
    w0a, w0r = g["w0a"].astype(np.float32), g["w0r"].astype(np.float32)
    for s in range(2):                  # seq-in-pair
        r0, c0 = s * CT, s * 64
        w0[r0:r0 + CT, 0, c0:c0 + 64] = w0a[:, :, 2].T          # tap2 on x
        w0[46 + r0:46 + r0 + CT, 0, c0:c0 + 64] = w0a[:, :, 1].T  # tap1, xsh
        w0[r0:r0 + CT, 1, c0:c0 + 64] = w0a[:, :, 0].T          # tap0
        w0[r0:r0 + CT, 2, c0:c0 + 64] = w0r[:, :, 0].T          # 1x1 resid

    # fp8 DoubleRow lhsT units [128, 2, 128]: DR#1 slots [tap1, tap2],
    # DR#2 slots [tap0, zero]
    def bd(w_tap):
        m = np.zeros((128, 128), np.float32)
        m[0:64, 0:64] = w_tap.T
        m[64:128, 64:128] = w_tap.T
        return m

    wk = np.zeros((128, 2 * NU, 128), np.float32)
    w0b = g["w0b"].astype(np.float32)
    wa, wb = g["wa"].astype(np.float32), g["wb"].astype(np.float32)

    def fill_conv(u, w):
        wk[:, 2 * u + 0, :] = bd(w[:, :, 1])
        wk[:, 2 * u + 1, :] = bd(w[:, :, 2])
        wk[:, 2 * u + 2, :] = bd(w[:, :, 0])
        # slot 2u+3 stays zero

    fill_conv(0, w0b)
    for i in range(4):
        fill_conv(2 + 4 * i, wa[i])
        fill_conv(2 + 4 * i + 2, wb[i])

    pet_w = g["pet_w"].astype(np.float32)[0, :, 0]    # (64,)
    pck_w = g["pck_w"].astype(np.float32)[0, :, 0]
    aet_w = g["aet_w"].astype(np.float32)[0, :, 0]    # (66,)
    wh = np.zeros((128, NPAIR, 96), np.float32)
    for p in range(NPAIR):
        wh[0:64, p, 4 * p + 0] = pet_w
        wh[64:128, p, 4 * p + 1] = pet_w
        wh[0:64, p, 4 * p + 2] = pck_w
        wh[64:128, p, 4 * p + 3] = pck_w
        wh[0:64, p, 64 + 4 * p + 0] = aet_w[0:64]
        wh[64:128, p, 64 + 4 * p + 1] = aet_w[0:64]
        wh[0:64, p, 64 + 4 * p + 2] = aet_w[0:64]
        wh[64:128, p, 64 + 4 * p + 3] = aet_w[0:64]
    wpet, wpck = float(aet_w[64]), float(aet_w[65])
    wsp = np.zeros((32, 32), np.float32)
    for p in range(NPAIR):
        r = 4 * p
        wsp[r + 0, r + 0] = wpet
        wsp[r + 1, r + 1] = wpet
        wsp[r + 2, r + 0] = wpck
        wsp[r + 3, r + 1] = wpck
        wsp[r + 0, r + 2] = wpet
        wsp[r + 1, r + 3] = wpet
        wsp[r + 2, r + 2] = wpck
        wsp[r + 3, r + 3] = wpck

    bcols = [g["b0a"], g["b0b"], g["b0r"]] + [g["ba"][i] for i in range(4)] \
        + [g["bb"][i] for i in range(4)]
    bias = np.stack([np.tile(c.astype(np.float32), 2) for c in bcols], axis=1)

    pet_b = float(g["pet_b"][0])
    pck_b = float(g["pck_b"][0])
    aet_b = float(g["aet_b"][0])
    bh = np.zeros((32, 2), np.float32)
    bh[:, 0] = np.tile([pet_b, pet_b, pck_b, pck_b], NPAIR)
    bh[:, 1] = -aet_b

    flags = (
        bool(np.all(bias[:, 0] == 0)),
        bool(np.all(bias[:, 1:3] == 0)),
        bool(np.all(bias[:, 3:7] == 0)),
        bool(np.all(bias[:, 7:11] == 0)),
        pet_b == 0.0 and pck_b == 0.0 and aet_b == 0.0,
    )
    shared = {"w0": w0, "wk": wk.astype(E4), "wh": wh,
              "wsp": wsp.astype(BF), "bias": bias, "bh": bh}
    return flags, shared, xin_cores


def run(inputs, trace=False, trace_kwargs=None, reps=1):
    flags, shared, xin_cores = prep_inputs(inputs)
    nc = get_program(flags, reps)
    in_maps = [
        {"xin": np.ascontiguousarray(xin_cores[c]), **shared}
        for c in range(NCORES)
    ]
    res = run_bass_kernel_spmd(nc, in_maps, core_ids=list(range(NCORES)),
                               trace=trace, **(trace_kwargs or {}))
    outs = []
    for nm in ("pet", "pck", "aet", "cwd"):
        full = np.concatenate(
            [np.asarray(res.results[c][nm]).astype(np.float32)
             for c in range(NCORES)], 0)
        outs.append(full.reshape(B, 1, T))
    return tuple(outs), res


def kernel(**inputs):
    outs, _ = run(inputs)
    return outs


def build_calib():
    """Same I/O signature as the real program, minimal compute — used by the
    bench to measure the axon relay's per-exec input-staging overhead."""
    _pin_act_table()
    nc = bacc.Bacc("TRN2", target_bir_lowering=False, debug=False,
                   num_devices=NCORES)
    xin_d = nc.dram_tensor("xin", [NPAIR, 2 * CT, M0 + T], F32R,
                           kind="ExternalInput")
    w0_d = nc.dram_tensor("w0", [4 * CT, 3, 128], F32R, kind="ExternalInput")
    wk_d = nc.dram_tensor("wk", [128, 2 * NU, 128], FP8, kind="ExternalInput")
    wh_d = nc.dram_tensor("wh", [128, NPAIR, 96], F32R,
                          kind="ExternalInput")
    wsp_d = nc.dram_tensor("wsp", [32, 32], BF16, kind="ExternalInput")
    bias_d = nc.dram_tensor("bias", [128, 11], F32, kind="ExternalInput")
    bh_d = nc.dram_tensor("bh", [32, 2], F32, kind="ExternalInput")
    out_d = {
        nm: nc.dram_tensor(nm, [BPC, T], BF16, kind="ExternalOutput")
        for nm in ("pet", "pck", "aet", "cwd")
    }
    with tile.TileContext(nc) as tc:
        with tc.tile_pool(name="sb", bufs=2) as sb:
            t = sb.tile([BPC, T], F32R)
            nc.sync.dma_start(out=t, in_=xin_d[0, 0:BPC, 0:T])
            o = sb.tile([BPC, T], BF16)
            nc.vector.tensor_scalar_mul(out=o, in0=t, scalar1=1.0)
            for nm in ("pet", "pck", "aet", "cwd"):
                nc.sync.dma_start(out=out_d[nm][:], in_=o)
    nc.compile()
    return nc


# revision 3
# speedup vs baseline: 1.5924x; 1.2624x over previous
"""Trainium2 Bass kernel for nn_BCMEmulator (TCN emulator) — fp8 DoubleRow rev.

Model: 5-block dilated-causal-conv TCN (CH=64, K=3, dils 1,2,4,8,16) over
(B=128, T=1024) + pointwise heads (pet/pck softplus, aet sigmoid gate, cwd).

Strategy (pure data parallel, 8 cores x 16 sequences; 2 seqs packed per
128 SBUF partitions = "pair", block-diagonal weights):
 - conv0a + 1x1 residual stay f32r (accuracy-critical first layer).
 - conv0b and all 4 residual blocks run in fp8-e4m3 with DoubleRow matmuls:
   one PE instruction contracts 2 k-tiles (2 conv taps) — taps are shifted
   views of a single zero-left-margin SBUF tile, expressed as a 3-dim access
   pattern [K, 2, N] whose middle dim strides by the dilation.
 - f (residual trunk) is kept as an exact f32r master (DVE relu+add update
   from PSUM); fp8 shadows for the next block's conv are cast on the
   otherwise-idle GPSIMD engine. Numerically validated: worst rel err
   ~1.1e-2 vs the 2e-2 gate (fp8 in block0's conv0b + blocks 1-4 only).
 - Heads are batched across all 8 pairs: per-pair 1x1 matmuls land in a
   shared (128,1024) PSUM tile (pet/pck logit rows 4p..4p+3, gate rows
   64+4p..); softplus/sigmoid chains then run as (32,1024) ACT passes
   (exp/ln only — single activation table) and (32,1024) DVE ops in bf16.
   Outputs ship as bf16 and are upcast on the host.
"""
import sys

sys.path.insert(0, "/opt/trn_rl_repo")

import numpy as np
import ml_dtypes

import concourse.bacc as bacc
import concourse.bass as bass
import concourse.tile as tile
from concourse import mybir
from concourse.bass_utils import run_bass_kernel_spmd

B, T = 128, 1024
C_IN, EMB = 15, 8
CH = 64
DILS = [1, 2, 4, 8, 16]
CT = C_IN + EMB              # 23 input channels after fveg concat
NCORES = 8
BPC = B // NCORES            # 16 sequences per core
NPAIR = BPC // 2             # 8 pairs per core
TT = 512                     # matmul free-dim tile (one PSUM bank of fp32)
NTT = T // TT
MARG = 32                    # left zero margin for block conv tiles (2*16)
M0 = 2                       # left zero margin for blk0 tiles

F32R = mybir.dt.float32r
F32 = mybir.dt.float32
FP8 = mybir.dt.float8e4
BF16 = mybir.dt.bfloat16
E4 = ml_dtypes.float8_e4m3
BF = ml_dtypes.bfloat16
AF = mybir.ActivationFunctionType
ALU = mybir.AluOpType
DRM = mybir.MatmulPerfMode.DoubleRow

NU = 18  # fp8 DR lhsT units: conv0b 2 + 4 blocks * (convA 2 + convB 2)

_PROGRAM_CACHE = {}


def _pin_act_table():
    """Force every ACT instruction onto natural_log_exp_and_others (contains
    Relu+Exp+Ln+Copy): avoids per-instruction table thrash (~2.7us a load)."""
    if getattr(bacc.get_activation_tables, "_pinned", False):
        return
    orig = bacc.get_activation_tables
    mine = {AF.Relu, AF.Exp, AF.Ln, AF.Copy}

    def patched(arch):
        tabs = orig(arch)
        return {
            name: (set(fns) if name == "natural_log_exp_and_others"
                   else set(fns) - mine)
            for name, fns in tabs.items()
        }

    patched._pinned = True
    bacc.get_activation_tables = patched


def dr_rhs(t_ap, base_col, d, n, k=128):
    """DoubleRow rhs AP [k, 2, n] on a 2-D tile: slot0 = cols shifted left by
    d (tap with shift d), slot1 = unshifted (tap with shift 0)."""
    pitch = t_ap.ap[0][0]
    return bass.AP(t_ap.tensor, t_ap.offset + base_col - d,
                   [[pitch, k], [d, 2], [1, n]])


def build_program(flags, reps=1):
    """flags: (zb0a, zb0b0r, zba, zbb, zheads) all-zero-bias fast paths."""
    _pin_act_table()
    zb0a, zb0b0r, zba, zbb, zheads = flags
    nc = bacc.Bacc("TRN2", target_bir_lowering=False, debug=False,
                   num_devices=NCORES)

    xin_d = nc.dram_tensor("xin", [NPAIR, 2 * CT, M0 + T], F32R,
                           kind="ExternalInput")
    w0_d = nc.dram_tensor("w0", [4 * CT, 3, 128], F32R, kind="ExternalInput")
    wk_d = nc.dram_tensor("wk", [128, 2 * NU, 128], FP8, kind="ExternalInput")
    wh_d = nc.dram_tensor("wh", [128, NPAIR, 96], F32R,
                          kind="ExternalInput")
    wsp_d = nc.dram_tensor("wsp", [32, 32], BF16, kind="ExternalInput")
    bias_d = nc.dram_tensor("bias", [128, 11], F32, kind="ExternalInput")
    bh_d = nc.dram_tensor("bh", [32, 2], F32, kind="ExternalInput")
    out_d = {
        nm: nc.dram_tensor(nm, [BPC, T], BF16, kind="ExternalOutput")
        for nm in ("pet", "pck", "aet", "cwd")
    }

    with tile.TileContext(nc) as tc:
        with (
            tc.tile_pool(name="wpool", bufs=1) as wpool,
            tc.tile_pool(name="xpool", bufs=1) as xpool,
            tc.tile_pool(name="hpool", bufs=1) as hpool,
            tc.tile_pool(name="fpool", bufs=1) as fpool,
            tc.tile_pool(name="spool", bufs=1) as spool,
            tc.tile_pool(name="pspool", bufs=1, space=bass.MemorySpace.PSUM) as ps,
        ):
            w0_sb = wpool.tile([4 * CT, 3, 128], F32R)
            wk_sb = wpool.tile([128, 2 * NU, 128], FP8)
            wh_sb = wpool.tile([128, NPAIR, 96], F32R)
            wsp_sb = wpool.tile([32, 32], BF16)
            bias_sb = wpool.tile([128, 11], F32)
            bh_sb = wpool.tile([32, 2], F32)
            nc.sync.dma_start(out=w0_sb, in_=w0_d[:])
            nc.sync.dma_start(out=wk_sb, in_=wk_d[:])
            nc.sync.dma_start(out=wh_sb, in_=wh_d[:])
            nc.sync.dma_start(out=wsp_sb, in_=wsp_d[:])
            nc.sync.dma_start(out=bias_sb, in_=bias_d[:])
            nc.sync.dma_start(out=bh_sb, in_=bh_d[:])

            # persistent data tiles
            xins = []
            for p in range(NPAIR):
                xt = xpool.tile([2 * 2 * CT, M0 + T], F32R, tag=f"xin{p}", name=f"xin_sb{p}")
                eng = nc.sync
                eng.dma_start(out=xt[0:2 * CT, :], in_=xin_d[p, :, :])
                # col 0 of the shifted rows is never read (all views start
                # at col >= M0-2 on rows 0:46 or >= M0 on the packed rows)
                eng.dma_start(out=xt[2 * CT:4 * CT, 1:M0 + T],
                              in_=xin_d[p, :, 0:M0 + T - 1])
                xins.append(xt)
            h0t = []
            for k in range(2):
                t = hpool.tile([128, M0 + T], FP8, tag=f"h0_{k}", name=f"h0t{k}")
                nc.vector.memset(t[:, 0:M0], 0.0)
                h0t.append(t)
            h1t = []
            for k in range(4):
                t = hpool.tile([128, MARG + T], FP8, tag=f"h1_{k}", name=f"h1t{k}")
                nc.vector.memset(t[:, 0:MARG], 0.0)
                h1t.append(t)
            sht = []
            for k in range(4):
                t = hpool.tile([128, MARG + T], FP8, tag=f"sh_{k}", name=f"sht{k}")
                nc.vector.memset(t[:, 0:MARG], 0.0)
                sht.append(t)
            fts = [fpool.tile([128, T], F32R, tag=f"f{p}", name=f"fm{p}")
                   for p in range(NPAIR)]

            psc = [ps.tile([128, T], F32, tag=f"pc{i}", name=f"pc{i}")
                   for i in range(3)]
            psH = ps.tile([128, T], F32, tag="ph", name="psHEAD")

            st = {}

            def blk0_convA(grp):
                for p in grp:
                    psA = psc[p % 2]
                    for t in range(NTT):
                        lo = t * TT
                        nc.tensor.matmul(psA[:, lo:lo + TT], w0_sb[:, 0, :],
                                         xins[p][0:4 * CT, M0 + lo:M0 + lo + TT],
                                         start=True, stop=False)
                        nc.tensor.matmul(psA[:, lo:lo + TT],
                                         w0_sb[0:2 * CT, 1, :],
                                         xins[p][0:2 * CT, lo:lo + TT],
                                         start=False, stop=True)
                    st[p] = {"ps": psA}

            def blk0_relu(grp):
                for p in grp:
                    h0 = h0t[p % 2]
                    if zb0a:
                        nc.scalar.activation(out=h0[:, M0:M0 + T],
                                             in_=st[p]["ps"], func=AF.Relu,
                                             bias=0.0, scale=1.0)
                    else:
                        nc.scalar.activation(out=h0[:, M0:M0 + T],
                                             in_=st[p]["ps"], func=AF.Relu,
                                             bias=bias_sb[:, 0:1], scale=1.0)
                    st[p]["h0"] = h0

            def blk0_convB(grp):
                for p in grp:
                    h0 = st[p]["h0"][:, :]
                    psB = psc[p % 2]
                    for t in range(NTT):
                        lo = t * TT
                        nc.tensor.matmul(psB[:, lo:lo + TT], wk_sb[:, 0:2, :],
                                         dr_rhs(h0, M0 + lo, 1, TT),
                                         start=True, stop=False,
                                         perf_mode=DRM)
                        nc.tensor.matmul(psB[:, lo:lo + TT], wk_sb[:, 2:4, :],
                                         dr_rhs(h0, M0 + lo, 2, TT),
                                         start=False, stop=True,
                                         perf_mode=DRM)
                    st[p]["ps"] = psB

            def blk0_resid(grp):
                for p in grp:
                    psR = psc[2]
                    for t in range(NTT):
                        lo = t * TT
                        nc.tensor.matmul(psR[:, lo:lo + TT],
                                         w0_sb[0:2 * CT, 2, :],
                                         xins[p][0:2 * CT, M0 + lo:M0 + lo + TT],
                                         start=True, stop=True)
                    st[p]["psR"] = psR

            def blk0_f0(grp):
                # walrus allows only one PSUM operand per DVE instruction:
                # materialize relu(psB)+b0b first, then add psR on DVE.
                # The relu runs on ACT or DVE per pair to balance the two
                # engines (ACT otherwise carries ~3us more than DVE).
                for p in grp:
                    f = fts[p]
                    h2 = spool.tile([128, T], F32R, tag=f"h2_{p % 2}", bufs=1,
                                    name=f"h2tmp{p % 2}")
                    b0b = 0.0 if zb0b0r else bias_sb[:, 1:2]
                    if p in (1, 3, 5):
                        nc.vector.tensor_scalar(out=h2, in0=st[p]["ps"],
                                                scalar1=b0b, scalar2=0.0,
                                                op0=ALU.add, op1=ALU.max)
                    else:
                        nc.scalar.activation(out=h2, in_=st[p]["ps"],
                                             func=AF.Relu, bias=b0b,
                                             scale=1.0)
                    if zb0b0r:
                        nc.vector.scalar_tensor_tensor(
                            out=f, in0=st[p]["psR"], scalar=0.0,
                            in1=h2, op0=ALU.add, op1=ALU.add)
                    else:
                        nc.vector.scalar_tensor_tensor(
                            out=f, in0=st[p]["psR"], scalar=bias_sb[:, 2:3],
                            in1=h2, op0=ALU.add, op1=ALU.add)
                    st[p]["f"] = f

            def cast(grp, k):
                for p in grp:
                    s = sht[2 * (p % 2) + k % 2]
                    nc.gpsimd.tensor_copy(out=s[:, MARG:MARG + T],
                                          in_=st[p]["f"])
                    st[p]["sh"] = s

            def blk_convA(grp, i, d):
                for p in grp:
                    sh = st[p]["sh"][:, :]
                    psA = psc[p % 2]
                    u = 2 * (2 + 4 * i)
                    for t in range(NTT):
                        lo = t * TT
                        nc.tensor.matmul(psA[:, lo:lo + TT],
                                         wk_sb[:, u:u + 2, :],
                                         dr_rhs(sh, MARG + lo, d, TT),
                                         start=True, stop=False,
                                         perf_mode=DRM)
                        nc.tensor.matmul(psA[:, lo:lo + TT],
                                         wk_sb[:, u + 2:u + 4, :],
                                         dr_rhs(sh, MARG + lo, 2 * d, TT),
                                         start=False, stop=True,
                                         perf_mode=DRM)
                    st[p]["ps"] = psA

            def blk_relu(grp, i):
                for p in grp:
                    h1 = h1t[2 * (p % 2) + i % 2]
                    if zba:
                        nc.scalar.activation(out=h1[:, MARG:MARG + T],
                                             in_=st[p]["ps"], func=AF.Relu,
                                             bias=0.0, scale=1.0)
                    else:
                        nc.scalar.activation(out=h1[:, MARG:MARG + T],
                                             in_=st[p]["ps"], func=AF.Relu,
                                             bias=bias_sb[:, 3 + i:4 + i],
                                             scale=1.0)
                    st[p]["h1"] = h1

            def blk_convB(grp, i, d):
                for p in grp:
                    h1 = st[p]["h1"][:, :]
                    psB = psc[p % 2]
                    u = 2 * (2 + 4 * i + 2)
                    for t in range(NTT):
                        lo = t * TT
                        nc.tensor.matmul(psB[:, lo:lo + TT],
                                         wk_sb[:, u:u + 2, :],
                                         dr_rhs(h1, MARG + lo, d, TT),
                                         start=True, stop=False,
                                         perf_mode=DRM)
                        nc.tensor.matmul(psB[:, lo:lo + TT],
                                         wk_sb[:, u + 2:u + 4, :],
                                         dr_rhs(h1, MARG + lo, 2 * d, TT),
                                         start=False, stop=True,
                                         perf_mode=DRM)
                    st[p]["ps"] = psB

            def blk_update(grp, i):
                for p in grp:
                    f = st[p]["f"]
                    if zbb:
                        nc.vector.scalar_tensor_tensor(
                            out=f, in0=st[p]["ps"], scalar=0.0,
                            in1=f.bitcast(F32), op0=ALU.max, op1=ALU.add)
                    else:
                        h2 = spool.tile([128, T], F32, tag="h2tmp", bufs=2, name="h2tmp")
                        nc.scalar.activation(out=h2, in_=st[p]["ps"],
                                             func=AF.Relu,
                                             bias=bias_sb[:, 7 + i:8 + i],
                                             scale=1.0)
                        nc.vector.tensor_tensor(out=f, in0=f.bitcast(F32),
                                                in1=h2, op=ALU.add)

            def head_mm(grp):
                # one accumulating matmul per pair per col-bank writes pet/pck
                # logit rows 0:32 AND gate logit rows 64:96 (lhsT zero rows
                # 32:64 between) — a single pending PSUM group per bank that
                # the tail's wsp matmul closes. Pair p's lhsT slice is zero
                # except out cols 4p..4p+3 (and 64+4p..).
                for p in grp:
                    f = st[p]["f"]
                    for t in range(NTT):
                        lo = t * TT
                        nc.tensor.matmul(psH[0:96, lo:lo + TT],
                                         wh_sb[:, p, :], f[:, lo:lo + TT],
                                         start=(p == 0),
                                         stop=(p == NPAIR - 1))

            for rep in range(reps):
                for g0 in range(0, NPAIR, 2):
                    grp = [g0, g0 + 1]
                    blk0_convA(grp)
                    blk0_relu(grp)
                    blk0_resid([grp[0]])
                    blk0_convB(grp)
                    blk0_f0([grp[0]])
                    blk0_resid([grp[1]])
                    blk0_f0([grp[1]])
                    cast(grp, 0)
                    for i, d in enumerate(DILS[1:]):
                        blk_convA(grp, i, d)
                        blk_relu(grp, i)
                        blk_convB(grp, i, d)
                        blk_update(grp, i)
                        if i < 3:
                            cast(grp, i + 1)
                    head_mm(grp)

                # ---- batched tail ----
                spe = spool.tile([32, T], BF16, tag="spe")
                sp = spool.tile([32, T], BF16, tag="sp")
                ge = spool.tile([32, T], F32, tag="ge")
                gl = spool.tile([32, T], F32, tag="gl")
                gg = spool.tile([32, T], F32, tag="gg")
                aet = spool.tile([32, T], BF16, tag="aet")
                cwd = spool.tile([32, T], BF16, tag="cwd")
                e1 = spool.tile([32, T], F32, tag="e1", name="e1t")
                for t in range(NTT):
                    sl = slice(t * TT, (t + 1) * TT)
                    if zheads:
                        nc.scalar.activation(out=spe[:, sl], in_=psH[0:32, sl],
                                             func=AF.Exp, bias=0.0, scale=1.0)
                    else:
                        nc.scalar.activation(out=spe[:, sl], in_=psH[0:32, sl],
                                             func=AF.Exp, bias=bh_sb[:, 0:1],
                                             scale=1.0)
                    nc.scalar.activation(out=sp[:, sl], in_=spe[:, sl],
                                         func=AF.Ln, bias=1.0, scale=1.0)
                    # gate logit is split: z1 = wha part (psH rows 64:96),
                    # z2 = rank-4 softplus part (free conv slot); only one
                    # PSUM operand is allowed per DVE op, so combine
                    # multiplicatively: exp(-(z1+z2)) = exp(-z1)*exp(-z2)
                    nc.tensor.matmul(psc[0][0:32, sl], wsp_sb, sp[0:32, sl],
                                     start=True, stop=True)
                    if zheads:
                        nc.scalar.activation(out=e1[:, sl],
                                             in_=psH[64:96, sl],
                                             func=AF.Exp, bias=0.0, scale=-1.0)
                    else:
                        nc.scalar.activation(out=e1[:, sl],
                                             in_=psH[64:96, sl],
                                             func=AF.Exp, bias=bh_sb[:, 1:2],
                                             scale=-1.0)
                    nc.scalar.activation(out=ge[:, sl], in_=psc[0][0:32, sl],
                                         func=AF.Exp, bias=0.0, scale=-1.0)
                    nc.vector.tensor_tensor(out=ge[:, sl], in0=ge[:, sl],
                                            in1=e1[:, sl], op=ALU.mult)
                    nc.scalar.activation(out=gl[:, sl], in_=ge[:, sl],
                                         func=AF.Ln, bias=1.0, scale=1.0)
                    nc.scalar.activation(out=gg[:, sl], in_=gl[:, sl],
                                         func=AF.Exp, scale=-1.0)
                    nc.vector.tensor_tensor(out=aet[:, sl], in0=gg[:, sl],
                                            in1=sp[:, sl], op=ALU.mult)
                    nc.vector.tensor_tensor(out=cwd[:, sl], in0=sp[:, sl],
                                            in1=aet[:, sl], op=ALU.subtract)
                for p in range(NPAIR):
                    nc.sync.dma_start(out=out_d["pet"][2 * p:2 * p + 2, :],
                                      in_=sp[4 * p:4 * p + 2, :])
                    nc.sync.dma_start(out=out_d["pck"][2 * p:2 * p + 2, :],
                                      in_=sp[4 * p + 2:4 * p + 4, :])
                    nc.sync.dma_start(out=out_d["aet"][2 * p:2 * p + 2, :],
                                      in_=aet[4 * p:4 * p + 2, :])
                    nc.sync.dma_start(out=out_d["cwd"][2 * p:2 * p + 2, :],
                                      in_=cwd[4 * p:4 * p + 2, :])

    nc.compile()
    return nc


def get_program(flags, reps=1):
    key = (tuple(flags), reps)
    if key not in _PROGRAM_CACHE:
        _PROGRAM_CACHE[key] = build_program(tuple(flags), reps)
    return _PROGRAM_CACHE[key]


def prep_inputs(inputs):
    """Host-side packing: returns (flags, shared weight map, per-core xin)."""
    g = {k: np.asarray(v) for k, v in inputs.items()}
    x = g["x"].astype(np.float32, copy=False)
    ids = g["fveg_ids"].astype(np.int64)
    emb = g["fveg_emb"].astype(np.float32, copy=False)

    fv = emb[ids]                                     # (B, EMB)
    xin = np.concatenate(
        [x, np.broadcast_to(fv[:, :, None], (B, EMB, T))], axis=1)  # (B,23,T)
    xin_pad = np.zeros((B, CT, M0 + T), np.float32)
    xin_pad[:, :, M0:] = xin
    xin_cores = np.ascontiguousarray(
        xin_pad.reshape(NCORES, NPAIR, 2 * CT, M0 + T))

    # f32r conv0a + resid weights: slot0 = taps(2,1) packed on 92 rows,
    # slot1 = tap0 (46 rows), slot2 = resid 1x1 (46 rows)
    w0 = np.zeros((4 * CT, 3, 128), np.float32)
    w0a, w0r = g["w0a"].astype(np.float32), g["w0r"].astype(np.float32)
    for s in range(2):                  # seq-in-pair
        r0, c0 = s * CT, s * 64
        w0[r0:r0 + CT, 0, c0:c0 + 64] = w0a[:, :, 2].T          # tap2 on x
        w0[46 + r0:46 + r0 + CT, 0, c0:c0 + 64] = w0a[:, :, 1].T  # tap1, xsh
        w0[r0:r0 + CT, 1, c0:c0 + 64] = w0a[:, :, 0].T          # tap0
        w0[r0:r0 + CT, 2, c0:c0 + 64] = w0r[:, :, 0].T          # 1x1 resid

    # fp8 DoubleRow lhsT units [128, 2, 128]: DR#1 slots [tap1, tap2],
    # DR#2 slots [tap0, zero]
    def bd(w_tap):
        m = np.zeros((128, 128), np.float32)
        m[0:64, 0:64] = w_tap.T
        m[64:128, 64:128] = w_tap.T
        return m

    wk = np.zeros((128, 2 * NU, 128), np.float32)
    w0b = g["w0b"].astype(np.float32)
    wa, wb = g["wa"].astype(np.float32), g["wb"].astype(np.float32)

    def fill_conv(u, w):
        wk[:, 2 * u + 0, :] = bd(w[:, :, 1])
        wk[:, 2 * u + 1, :] = bd(w[:, :, 2])
        wk[:, 2 * u + 2, :] = bd(w[:, :, 0])
        # slot 2u+3 stays zero

    fill_conv(0, w0b)
    for i in range(4):
        fill_conv(2 + 4 * i, wa[i])
        fill_conv(2 + 4 * i + 2, wb[i])

    pet_w = g["pet_w"].astype(np.float32)[0, :, 0]    # (64,)
    pck_w = g["pck_w"].astype(np.float32)[0, :, 0]
    aet_w = g["aet_w"].astype(np.float32)[0, :, 0]    # (66,)
    wh = np.zeros((128, NPAIR, 96), np.float32)
    for p in range(NPAIR):
        wh[0:64, p, 4 * p + 0] = pet_w
        wh[64:128, p, 4 * p + 1] = pet_w
        wh[0:64, p, 4 * p + 2] = pck_w
        wh[64:128, p, 4 * p + 3] = pck_w
        wh[0:64, p, 64 + 4 * p + 0] = aet_w[0:64]
        wh[64:128, p, 64 + 4 * p + 1] = aet_w[0:64]
        wh[0:64, p, 64 + 4 * p + 2] = aet_w[0:64]
        wh[64:128, p, 64 + 4 * p + 3] = aet_w[0:64]
    wpet, wpck = float(aet_w[64]), float(aet_w[65])
    wsp = np.zeros((32, 32), np.float32)
    for p in range(NPAIR):
        r = 4 * p
        wsp[r + 0, r + 0] = wpet
        wsp[r + 1, r + 1] = wpet
        wsp[r + 2, r + 0] = wpck
        wsp[r + 3, r + 1] = wpck
        wsp[r + 0, r + 2] = wpet
        wsp[r + 1, r + 3] = wpet
        wsp[r + 2, r + 2] = wpck
        wsp[r + 3, r + 3] = wpck

    bcols = [g["b0a"], g["b0b"], g["b0r"]] + [g["ba"][i] for i in range(4)] \
        + [g["bb"][i] for i in range(4)]
    bias = np.stack([np.tile(c.astype(np.float32), 2) for c in bcols], axis=1)

    pet_b = float(g["pet_b"][0])
    pck_b = float(g["pck_b"][0])
    aet_b = float(g["aet_b"][0])
    bh = np.zeros((32, 2), np.float32)
    bh[:, 0] = np.tile([pet_b, pet_b, pck_b, pck_b], NPAIR)
    bh[:, 1] = -aet_b

    flags = (
        bool(np.all(bias[:, 0] == 0)),
        bool(np.all(bias[:, 1:3] == 0)),
        bool(np.all(bias[:, 3:7] == 0)),
        bool(np.all(bias[:, 7:11] == 0)),
        pet_b == 0.0 and pck_b == 0.0 and aet_b == 0.0,
    )
    shared = {"w0": w0, "wk": wk.astype(E4), "wh": wh,
              "wsp": wsp.astype(BF), "bias": bias, "bh": bh}
    return flags, shared, xin_cores


def run(inputs, trace=False, trace_kwargs=None, reps=1):
    flags, shared, xin_cores = prep_inputs(inputs)
    nc = get_program(flags, reps)
    in_maps = [
        {"xin": np.ascontiguousarray(xin_cores[c]), **shared}
        for c in range(NCORES)
    ]
    res = run_bass_kernel_spmd(nc, in_maps, core_ids=list(range(NCORES)),
                               trace=trace, **(trace_kwargs or {}))
    outs = []
    for nm in ("pet", "pck", "aet", "cwd"):
        full = np.concatenate(
            [np.asarray(res.results[c][nm]).astype(np.float32)
             for c in range(NCORES)], 0)
        outs.append(full.reshape(B, 1, T))
    return tuple(outs), res


def kernel(**inputs):
    outs, _ = run(inputs)
    return outs


def build_calib():
    """Same I/O signature as the real program, minimal compute — used by the
    bench to measure the axon relay's per-exec input-staging overhead."""
    _pin_act_table()
    nc = bacc.Bacc("TRN2", target_bir_lowering=False, debug=False,
                   num_devices=NCORES)
    xin_d = nc.dram_tensor("xin", [NPAIR, 2 * CT, M0 + T], F32R,
                           kind="ExternalInput")
    w0_d = nc.dram_tensor("w0", [4 * CT, 3, 128], F32R, kind="ExternalInput")
    wk_d = nc.dram_tensor("wk", [128, 2 * NU, 128], FP8, kind="ExternalInput")
    wh_d = nc.dram_tensor("wh", [128, NPAIR, 96], F32R,
                          kind="ExternalInput")
    wsp_d = nc.dram_tensor("wsp", [32, 32], BF16, kind="ExternalInput")
    bias_d = nc.dram_tensor("bias", [128, 11], F32, kind="ExternalInput")
    bh_d = nc.dram_tensor("bh", [32, 2], F32, kind="ExternalInput")
    out_d = {
        nm: nc.dram_tensor(nm, [BPC, T], BF16, kind="ExternalOutput")
        for nm in ("pet", "pck", "aet", "cwd")
    }
    with tile.TileContext(nc) as tc:
        with tc.tile_pool(name="sb", bufs=2) as sb:
            t = sb.tile([BPC, T], F32R)
            nc.sync.dma_start(out=t, in_=xin_d[0, 0:BPC, 0:T])
            o = sb.tile([BPC, T], BF16)
            nc.vector.tensor_scalar_mul(out=o, in0=t, scalar1=1.0)
            for nm in ("pet", "pck", "aet", "cwd"):
                nc.sync.dma_start(out=out_d[nm][:], in_=o)
    nc.compile()
    return nc
